# revision 1
# baseline (speedup 1.0000x reference)
"""CompressionTransformer Trainium2 kernel.

Sharding (8 cores): core c -> batch b = c//4, vocab column block j = c%4.
Each core computes the full transformer body for its batch (1024 tokens)
and the final vocab projection for its 8000-column slice.  No collectives.

On-device layout: activations are kept feature-major ("transposed"):
xT[d, s] with d on partitions (tiles of 128) and tokens on the free dim.

Attention core = chunked linear attention over S in blocks of 128:
  state[d, c] = sum_{t < block} w[c,t] * vkv[t, d]   (SBUF accumulator)
with within-block terms computed via triangular-masked matmuls, and the
softmax normalizer cumsum done with the DVE prefix-scan instruction.

Hardware constraint honored throughout: compute-engine operands must
share the same partition range (walrus `samePartitionsAll`); the only
partition-base shifts are done with DMA (head-1 projection split) or by
matmul PSUM output placement (odd-head attention output at base 64).
"""

import numpy as np
from contextlib import ExitStack

import concourse.bass as bass
import concourse.mybir as mybir
import concourse.tile as tile
from concourse import bacc
from concourse.bass_utils import run_bass_kernel_spmd
from concourse.masks import make_identity, make_upper_triangular

F32 = mybir.dt.float32
AX = mybir.AluOpType
AF = mybir.ActivationFunctionType
AXL = mybir.AxisListType

B, S, D, H, C, V, MLP, OUT = 2, 1024, 512, 8, 64, 32000, 1024, 32000
DH = D // H          # 64
ROT = DH // 2        # 32
EPS = 1e-5
THETA = 10000.0
P = 128
KD = D // P          # 4  k-tiles over D
KM = MLP // P        # 8  k-tiles over MLP
NCORE = 8
VSL = OUT // 4       # 8000 per-core vocab slice
BLK = 128
NB = S // BLK        # 8
SCALE = DH ** -0.5
VCH = 500            # vocab columns per psum tile
NCH = VSL // (2 * VCH)  # 8 chunks of 1000 columns

_LN_ID = [0]


def _layer_norm(nc, tc, ps512, ones_col, ones_row, eps_ap, src, dst,
                g_cols, b_cols, r_tiles):
    """dst = LN(src) * g + b over the partition-tiled feature dim.
    src/dst: (128, r_tiles, 1024) feature-major. Scratch lives in a pool
    scoped to this call so LN1/LN2/LNf reuse the same SBUF."""
    _LN_ID[0] += 1
    with tc.tile_pool(name=f"lnp{_LN_ID[0]}", bufs=1) as lnp:
        sumx = lnp.tile([1, S], F32, name="sumx")
        sumx2 = lnp.tile([1, S], F32, name="sumx2")
        tmp = lnp.tile([1, S], F32, name="lntmp")
        rstd = lnp.tile([1, S], F32, name="rstd")
        nmr = lnp.tile([1, S], F32, name="nmr")
        nc.vector.memset(sumx[:], 0.0)
        nc.vector.memset(sumx2[:], 0.0)
        for r in range(r_tiles):
            sq = lnp.tile([P, S], F32, tag="lnsq", name=f"lnsq{r}", bufs=2)
            nc.scalar.square(sq[:], src[:, r, :])
            for hh in range(2):
                sl = slice(hh * 512, hh * 512 + 512)
                ps_a = ps512.tile([1, 512], F32, tag="ps512",
                                  name=f"lps{r}{hh}a")
                nc.tensor.matmul(ps_a[:], ones_col[:], src[:, r, sl],
                                 start=True, stop=True)
                nc.vector.tensor_add(sumx[:, sl], sumx[:, sl], ps_a[:])
                ps_b = ps512.tile([1, 512], F32, tag="ps512",
                                  name=f"lps{r}{hh}b")
                nc.tensor.matmul(ps_b[:], ones_col[:], sq[:, sl],
                                 start=True, stop=True)
                nc.vector.tensor_add(sumx2[:, sl], sumx2[:, sl], ps_b[:])
        dd = float(r_tiles * P)
        nc.scalar.mul(sumx[:], sumx[:], 1.0 / dd)    # mean
        nc.scalar.mul(sumx2[:], sumx2[:], 1.0 / dd)  # E[x^2]
        nc.vector.tensor_mul(tmp[:], sumx[:], sumx[:])
        nc.vector.tensor_sub(tmp[:], sumx2[:], tmp[:])        # var
        nc.scalar.activation(tmp[:], tmp[:], AF.Sqrt, bias=eps_ap)
        nc.vector.reciprocal(rstd[:], tmp[:])
        nc.vector.scalar_tensor_tensor(nmr[:], sumx[:], -1.0, rstd[:],
                                       op0=AX.mult, op1=AX.mult)
        for hh in range(2):
            sl = slice(hh * 512, hh * 512 + 512)
            ps_r = ps512.tile([P, 512], F32, tag="ps512",
                              name=f"lbr{hh}")
            ps_n = ps512.tile([P, 512], F32, tag="ps512",
                              name=f"lbn{hh}")
            nc.tensor.matmul(ps_r[:], ones_row[:], rstd[:, sl], start=True,
                             stop=True)
            nc.tensor.matmul(ps_n[:], ones_row[:], nmr[:, sl], start=True,
                             stop=True)
            for r in range(r_tiles):
                t = dst[:, r, sl]
                nc.vector.tensor_mul(t, src[:, r, sl], ps_r[:])
                nc.vector.tensor_add(t, t, ps_n[:])
        for r in range(r_tiles):
            nc.scalar.activation(dst[:, r, :], dst[:, r, :], AF.Identity,
                                 bias=b_cols[:, r:r + 1],
                                 scale=g_cols[:, r:r + 1])


def _proj(nc, ps512, w_tile, rhs_tile, dst, bias_cols, m_tiles, k_tiles,
          resid=None):
    """dst (128, m_tiles, 1024) = W.T @ rhs + b [+ resid], feature-major."""
    for m in range(m_tiles):
        for hh in range(2):
            sl = slice(hh * 512, hh * 512 + 512)
            ps = ps512.tile([P, 512], F32, tag="ps512",
                            name=f"prj{m}{hh}")
            for k in range(k_tiles):
                nc.tensor.matmul(ps[:], w_tile[:, k, m * P:(m + 1) * P],
                                 rhs_tile[:, k, sl], start=(k == 0),
                                 stop=(k == k_tiles - 1))
            if resid is None:
                nc.scalar.activation(dst[:, m, sl], ps[:], AF.Identity,
                                     bias=bias_cols[:, m:m + 1], scale=1.0)
            else:
                nc.vector.scalar_tensor_tensor(
                    dst[:, m, sl], ps[:], bias_cols[:, m:m + 1],
                    resid[:, m, sl], op0=AX.add, op1=AX.add)


def _attn_head(nc, hsc, sb, ps512, pta, ptsm, ident, ut, qdT_s,
               qu_p, kd_0, vk_p, vv_p, po, hd, outT):
    """One compression-attention head.  qu/vk/vv are used as pair-tile
    slices at partition base po (matmul inputs only -- their PSUM results
    land at base 0); kd_0 is a base-0 (64, S) view/copy (rope needs
    elementwise ops against PSUM outputs, which are always base 0).
    All other per-head tensors live at base 0.  For po=64 the K/V column
    order inside vkv_tok/state is swapped so stateK sits at rows
    [64:128], matching qu's partition base in the logits matmul."""
    hsl = slice(po, po + DH)             # this head's rows in pair tiles
    kcol = slice(po, po + DH)            # K half inside (t, d) tiles
    vcol = slice((po + DH) % P, (po + DH) % P + DH)  # V half
    idq = ident[hsl, hsl]                # 64x64 identity at base po
    id0 = ident[0:DH, 0:DH]
    # down[c, s] = qd_scaled[:, c] . kd_rope[:, s]
    down_ps = ps512.tile([C, S], F32, tag="pdown", bufs=1, name=f"dwn{hd}")
    for hh in range(2):
        sl = slice(hh * 512, hh * 512 + 512)
        nc.tensor.matmul(down_ps[:, sl], qdT_s[:, hd * C:(hd + 1) * C],
                         kd_0[:, sl], start=True, stop=True)
    mx = sb.tile([C, 1], F32, tag="amax", name=f"amax{hd}")
    nc.vector.reduce_max(mx[:], down_ps[:], axis=AXL.X)
    nmx = sb.tile([C, 1], F32, tag="anmax", name=f"anmax{hd}")
    nc.vector.tensor_scalar_mul(nmx[:], mx[:], -1.0)
    w_cs = hsc.tile([C, S], F32, tag="wcs", name=f"wcs{hd}")
    nc.scalar.activation(w_cs[:], down_ps[:], AF.Exp, bias=nmx[:],
                         scale=1.0)
    # inclusive cumsum over s: state=(w+state) max w  (w>0 so max is id)
    ncs = hsc.tile([C, S], F32, tag="ncs", name=f"ncs{hd}")
    nc.vector.tensor_tensor_scan(ncs[:], w_cs[:], w_cs[:], 0.0,
                                 op0=AX.add, op1=AX.max)
    nc.vector.reciprocal(ncs[:], ncs[:])  # in-place: 1/cumsum

    state = sb.tile([P, C], F32, tag="state", name=f"st{hd}")  # (2DH, C)
    nc.vector.memset(state[:], 0.0)

    otile = (hd * DH) // P
    if po:
        oT_h = hsc.tile([DH, S], F32, tag="oTh", name=f"oTh{hd}")

    for n in range(NB):
        bsl = slice(n * BLK, (n + 1) * BLK)
        # vkv_tok (t, d): transposed vk/vv block halves (K at kcol)
        vkvP = pta.tile([P, P], F32, tag="pta", name=f"vkvP{hd}_{n}")
        nc.tensor.transpose(vkvP[:, kcol], vk_p[hsl, bsl], idq)
        nc.tensor.transpose(vkvP[:, vcol], vv_p[hsl, bsl], idq)
        vkv_tok = sb.tile([P, P], F32, tag="vkvtok", name=f"vkv{hd}_{n}")
        nc.scalar.copy(vkv_tok[:], vkvP[:])
        # w_tok (t, c)
        wtP = ptsm.tile([P, C], F32, tag="ptsm", name=f"wtP{hd}_{n}")
        nc.tensor.transpose(wtP[:], w_cs[:, bsl], id0)
        w_tok = sb.tile([P, C], F32, tag="wtok", name=f"wtk{hd}_{n}")
        nc.vector.tensor_copy(w_tok[:], wtP[:])
        # invn_T (t, c)
        invP = ptsm.tile([P, C], F32, tag="ptsm", name=f"invP{hd}_{n}")
        nc.tensor.transpose(invP[:], ncs[:, bsl], id0)
        invn_T = sb.tile([P, C], F32, tag="invnT", name=f"invT{hd}_{n}")
        nc.scalar.copy(invn_T[:], invP[:])
        # A_T[t,i] = vk[t] . qu[i], keep t <= i
        atP = pta.tile([P, P], F32, tag="pta", name=f"atP{hd}_{n}")
        nc.tensor.matmul(atP[:], vk_p[hsl, bsl], qu_p[hsl, bsl],
                         start=True, stop=True)
        at_m = sb.tile([P, P], F32, tag="atm", name=f"atm{hd}_{n}")
        nc.vector.tensor_mul(at_m[:], atP[:], ut[:])
        # state snapshot (c, d) at base 0
        cdP = ptsm.tile([C, P], F32, tag="ptsm", name=f"cdP{hd}_{n}")
        nc.tensor.transpose(cdP[:], state[:], ident[:])
        st_cd = sb.tile([C, P], F32, tag="stcd", name=f"stcd{hd}_{n}")
        nc.scalar.copy(st_cd[:], cdP[:])
        # logits(i, c) = qu . stateK + within-block
        lgP = ptsm.tile([P, C], F32, tag="ptsm", name=f"lgP{hd}_{n}")
        nc.tensor.matmul(lgP[:], qu_p[hsl, bsl], state[kcol, :],
                         start=True, stop=False)
        nc.tensor.matmul(lgP[:], at_m[:], w_tok[:], start=False, stop=True)
        lg = sb.tile([P, C], F32, tag="lg", name=f"lg{hd}_{n}")
        nc.vector.scalar_tensor_tensor(lg[:], lgP[:], SCALE, invn_T[:],
                                       op0=AX.mult, op1=AX.mult)
        # softmax over c (free dim)
        smx = sb.tile([P, 1], F32, tag="smax", name=f"smx{hd}_{n}")
        nc.vector.reduce_max(smx[:], lg[:], axis=AXL.X)
        nsmx = sb.tile([P, 1], F32, tag="snmax", name=f"nsmx{hd}_{n}")
        nc.vector.tensor_scalar_mul(nsmx[:], smx[:], -1.0)
        pexp = sb.tile([P, C], F32, tag="pexp", name=f"pex{hd}_{n}")
        ssum = sb.tile([P, 1], F32, tag="ssum", name=f"ssum{hd}_{n}")
        nc.scalar.activation(pexp[:], lg[:], AF.Exp, bias=nsmx[:],
                             scale=1.0, accum_out=ssum[:])
        rsum = sb.tile([P, 1], F32, tag="rsum", name=f"rsum{hd}_{n}")
        nc.vector.reciprocal(rsum[:], ssum[:])
        r_sb = sb.tile([P, C], F32, tag="rsb", name=f"rsb{hd}_{n}")
        nc.vector.tensor_scalar_mul(r_sb[:], pexp[:], rsum[:])
        nc.vector.tensor_mul(r_sb[:], r_sb[:], invn_T[:])
        # rT (c, i) at base 0
        rtP = ptsm.tile([C, P], F32, tag="ptsm", name=f"rtP{hd}_{n}")
        nc.tensor.transpose(rtP[:], r_sb[:], ident[:])
        rT = sb.tile([C, P], F32, tag="rt", name=f"rT{hd}_{n}")
        nc.scalar.copy(rT[:], rtP[:])
        # G_T[t,i] = sum_c w[c,t] rT[c,i], keep t <= i
        gtP = pta.tile([P, P], F32, tag="pta", name=f"gtP{hd}_{n}")
        nc.tensor.matmul(gtP[:], w_cs[:, bsl], rT[:], start=True,
                         stop=True)
        gt_m = sb.tile([P, P], F32, tag="gtm", name=f"gtm{hd}_{n}")
        nc.vector.tensor_mul(gt_m[:], gtP[:], ut[:])
        # out block (dv, i) at psum base 0
        outP = ptsm.tile([DH, P], F32, tag="ptsm", name=f"outP{hd}_{n}")
        nc.tensor.matmul(outP[:], st_cd[:, vcol], rT[:], start=True,
                         stop=False)
        nc.tensor.matmul(outP[:], vkv_tok[:, vcol], gt_m[:], start=False,
                         stop=True)
        if po:
            nc.scalar.copy(oT_h[:, bsl], outP[:])
        else:
            nc.scalar.copy(outT[0:DH, otile, bsl], outP[:])
        # state += vkv_tok.T @ w_tok
        sdP = pta.tile([P, C], F32, tag="pta", name=f"sdP{hd}_{n}")
        nc.tensor.matmul(sdP[:], vkv_tok[:], w_tok[:], start=True,
                         stop=True)
        nc.vector.tensor_add(state[:], state[:], sdP[:])

    if po:
        # partition shift rows [0:64] -> [64:128] via SBUF-to-SBUF DMA
        nc.sync.dma_start(outT[DH:P, otile, :], oT_h[:])


def _body(nc, tc, ctx, t):
    x0T, Wq, Wk, Wv, Wo = t["x0T"], t["Wq"], t["Wk"], t["Wv"], t["Wo"]
    W1, W2, WoutS, out_d = t["W1"], t["W2"], t["WoutS"], t["out_d"]
    qdT, cosT, sinT, RT = t["qdT"], t["cosT"], t["sinT"], t["RT"]

    const = ctx.enter_context(tc.tile_pool(name="const", bufs=1))
    acts = ctx.enter_context(tc.tile_pool(name="acts", bufs=1))
    sb = ctx.enter_context(tc.tile_pool(name="sb", bufs=2))

    # ---- constants ----
    ident = const.tile([P, P], F32)
    make_identity(nc, ident[:])
    ut = const.tile([P, P], F32)
    make_upper_triangular(nc, ut[:], val=1.0, diag=True)
    ones_col = const.tile([P, 1], F32)
    nc.vector.memset(ones_col[:], 1.0)
    ones_row = const.tile([1, P], F32)
    nc.vector.memset(ones_row[:], 1.0)
    eps_t = const.tile([1, 1], F32)
    nc.vector.memset(eps_t[:], EPS)

    cosT_t = const.tile([ROT, S], F32)
    nc.sync.dma_start(cosT_t[:], cosT)
    sinT_t = const.tile([ROT, S], F32)
    nc.sync.dma_start(sinT_t[:], sinT)
    RT_t = const.tile([ROT, ROT], F32)
    nc.sync.dma_start(RT_t[:], RT)
    qdT_s = const.tile([DH, H * C], F32)
    nc.sync.dma_start(qdT_s[:], qdT)
    nc.scalar.mul(qdT_s[:], qdT_s[:], SCALE)

    def load_col(ap, kt, name, p=P):
        c_ = const.tile([p, kt], F32, name=name)
        nc.sync.dma_start(c_[:], ap.rearrange("(t p) o -> p (t o)", p=p))
        return c_

    # full-width (128 x KD) bias/gain columns
    bo_c = load_col(t["bo"], KD, "bo_c")
    b1_c = load_col(t["b1"], KM, "b1_c")
    b2_c = load_col(t["b2"], KD, "b2_c")
    ln1g_c = load_col(t["ln1g"], KD, "ln1g_c")
    ln1b_c = load_col(t["ln1b"], KD, "ln1b_c")
    ln2g_c = load_col(t["ln2g"], KD, "ln2g_c")
    ln2b_c = load_col(t["ln2b"], KD, "ln2b_c")
    lnfg_c = load_col(t["lnfg"], KD, "lnfg_c")
    lnfb_c = load_col(t["lnfb"], KD, "lnfb_c")
    bv_c = load_col(t["bv"], KD, "bv_c")
    bq_c = load_col(t["bq"], KD, "bq_c")
    bk_c = load_col(t["bk"], KD, "bk_c")

    x0_t = acts.tile([P, KD, S], F32, tag="A")
    nc.sync.dma_start(x0_t[:], x0T.rearrange("(t p) n -> p t n", p=P))

    with tc.tile_pool(name="wmain", bufs=1) as wmain:
        def load_w(ap, kt, n, name):
            w = wmain.tile([P, kt, n], F32, name=name)
            nc.sync.dma_start(w[:], ap.rearrange("(t p) n -> p t n", p=P))
            return w

        Wq_t = load_w(Wq, KD, D, "Wq_t")
        Wk_t = load_w(Wk, KD, D, "Wk_t")
        Wv_t = load_w(Wv, KD, D, "Wv_t")
        Wo_t = load_w(Wo, KD, D, "Wo_t")

        with tc.tile_pool(name="ps512", bufs=2, space="PSUM") as ps512:
            # ---- LN1 ----
            h_t = acts.tile([P, KD, S], F32, tag="B")
            _layer_norm(nc, tc, ps512, ones_col, ones_row, eps_t[:], x0_t,
                        h_t, ln1g_c, ln1b_c, KD)

            # ---- vd = h @ Wv + bv ----
            vd_t = acts.tile([P, KD, S], F32, tag="C")
            _proj(nc, ps512, Wv_t, h_t, vd_t, bv_c, KD, KD)

            outT = acts.tile([P, KD, S], F32, tag="A")

            # ---- head pairs: projections + rope + attention ----
            with tc.tile_pool(name="hsc", bufs=2) as hsc, \
                 tc.tile_pool(name="pta", bufs=2, space="PSUM") as pta, \
                 tc.tile_pool(name="ptsm", bufs=2, space="PSUM") as ptsm:
                for pr in range(4):
                    msl = slice(pr * P, (pr + 1) * P)
                    ht = {k: hsc.tile([P, S], F32, tag=f"hd_{k}",
                                      name=f"{k}_{pr}")
                          for k in ("qu", "kd", "vk", "vv")}
                    for ky, wt, bcol, rhs in (("qu", Wq_t, bq_c, h_t),
                                              ("kd", Wk_t, bk_c, h_t),
                                              ("vk", Wk_t, bk_c, vd_t),
                                              ("vv", Wv_t, bv_c, vd_t)):
                        for hh in range(2):
                            sl = slice(hh * 512, hh * 512 + 512)
                            ps = ps512.tile([P, 512], F32, tag="ps512",
                                            name=f"pp{pr}{ky}{hh}")
                            for k in range(KD):
                                nc.tensor.matmul(ps[:], wt[:, k, msl],
                                                 rhs[:, k, sl],
                                                 start=(k == 0),
                                                 stop=(k == KD - 1))
                            if hh % 2:
                                nc.scalar.activation(
                                    ht[ky][:, sl], ps[:], AF.Identity,
                                    bias=bcol[:, pr:pr + 1], scale=1.0)
                            else:
                                nc.vector.tensor_scalar_add(
                                    ht[ky][:, sl], ps[:],
                                    bcol[:, pr:pr + 1])
                    # per-head base-0 kd (odd head: DMA partition shift),
                    # then rope on its first ROT dims
                    kd0s = []
                    for hi in range(2):
                        if hi == 0:
                            kd_0 = ht["kd"][0:DH, :]
                        else:
                            kd_0t = hsc.tile([DH, S], F32, tag="kd0",
                                             name=f"kd0_{pr}")
                            nc.sync.dma_start(kd_0t[:], ht["kd"][DH:P, :])
                            kd_0 = kd_0t[:]
                        kd0s.append(kd_0)
                        tmp = hsc.tile([ROT, S], F32, tag="ropet",
                                       name=f"ropes_{pr}_{hi}")
                        nc.vector.tensor_mul(tmp[:], kd_0[0:ROT, :],
                                             sinT_t[:])
                        rp = ps512.tile([ROT, 512], F32, tag="ps512",
                                        name=f"rp{pr}{hi}")
                        rp2 = ps512.tile([ROT, 512], F32, tag="ps512",
                                         name=f"rq{pr}{hi}")
                        nc.tensor.matmul(rp[:], RT_t[:], tmp[:, 0:512],
                                         start=True, stop=True)
                        nc.tensor.matmul(rp2[:], RT_t[:], tmp[:, 512:1024],
                                         start=True, stop=True)
                        tmp2 = hsc.tile([ROT, S], F32, tag="ropet",
                                        name=f"ropec_{pr}_{hi}")
                        nc.vector.tensor_mul(tmp2[:], kd_0[0:ROT, :],
                                             cosT_t[:])
                        nc.vector.tensor_add(kd_0[0:ROT, 0:512],
                                             tmp2[:, 0:512], rp[:])
                        nc.vector.tensor_add(kd_0[0:ROT, 512:1024],
                                             tmp2[:, 512:1024], rp2[:])

                    for hi in range(2):
                        _attn_head(nc, hsc, sb, ps512, pta, ptsm, ident,
                                   ut, qdT_s, ht["qu"], kd0s[hi],
                                   ht["vk"], ht["vv"], hi * DH,
                                   2 * pr + hi, outT)

            # ---- a = out @ Wo + bo ; resid = h + a ----
            resid = acts.tile([P, KD, S], F32, tag="C")
            _proj(nc, ps512, Wo_t, outT, resid, bo_c, KD, KD, resid=h_t)

            # ---- x2 = LN2(resid) ----
            x2_t = acts.tile([P, KD, S], F32, tag="A")
            _layer_norm(nc, tc, ps512, ones_col, ones_row, eps_t[:], resid,
                        x2_t, ln2g_c, ln2b_c, KD)

            # ---- MLP: x3 = x2 + relu(x2@W1+b1)@W2 + b2 ----
            x3_t = acts.tile([P, KD, S], F32, tag="B")
            w1_r = W1.rearrange("(t p) n -> p t n", p=P)
            with tc.tile_pool(name="wmlp", bufs=2) as wmlp, \
                 tc.tile_pool(name="psacc", bufs=4, space="PSUM") as psacc:
                for hh in range(2):
                    sl = slice(hh * 512, hh * 512 + 512)
                    m2ps = [psacc.tile([P, 512], F32, tag="m2ps",
                                       name=f"m2ps_{hh}_{i}")
                            for i in range(KD)]
                    for km in range(KM):
                        w1t = wmlp.tile([P, KD, P], F32, tag="w1t",
                                        name=f"w1t_{hh}_{km}")
                        nc.sync.dma_start(
                            w1t[:], w1_r[:, :, km * P:(km + 1) * P])
                        w2t = wmlp.tile([P, D], F32, tag="w2t",
                                        name=f"w2t_{hh}_{km}")
                        nc.sync.dma_start(w2t[:],
                                          W2[km * P:(km + 1) * P, :])
                        m1ps = ps512.tile([P, 512], F32, tag="ps512",
                                          name=f"m1ps_{hh}_{km}")
                        for k in range(KD):
                            nc.tensor.matmul(m1ps[:], w1t[:, k, :],
                                             x2_t[:, k, sl], start=(k == 0),
                                             stop=(k == KD - 1))
                        m1sb = sb.tile([P, 512], F32, tag="m1sb",
                                       name=f"m1sb_{hh}_{km}")
                        nc.scalar.activation(m1sb[:], m1ps[:], AF.Relu,
                                             bias=b1_c[:, km:km + 1],
                                             scale=1.0)
                        for m in range(KD):
                            nc.tensor.matmul(m2ps[m][:],
                                             w2t[:, m * P:(m + 1) * P],
                                             m1sb[:], start=(km == 0),
                                             stop=(km == KM - 1))
                    for m in range(KD):
                        nc.vector.scalar_tensor_tensor(
                            x3_t[:, m, sl], m2ps[m][:], b2_c[:, m:m + 1],
                            x2_t[:, m, sl], op0=AX.add, op1=AX.add)

            # ---- xf = LNf(x3) ----
            xf_t = acts.tile([P, KD, S], F32, tag="C")
            _layer_norm(nc, tc, ps512, ones_col, ones_row, eps_t[:], x3_t,
                        xf_t, lnfg_c, lnfb_c, KD)

    # ---- final projection: out = xf @ WoutS ----
    with tc.tile_pool(name="wout", bufs=2) as woutp, \
         tc.tile_pool(name="stg", bufs=3) as stgp, \
         tc.tile_pool(name="psf", bufs=4, space="PSUM") as psf:
        wout_r = WoutS.rearrange("(t p) n -> p t n", p=P)
        cw = 2 * VCH
        for ch in range(NCH):
            c0 = ch * cw
            wt = woutp.tile([P, KD, cw], F32, tag="woutt",
                            name=f"wout_{ch}")
            nc.sync.dma_start(wt[:], wout_r[:, :, c0:c0 + cw])
            for m in range(8):
                stg_t = stgp.tile([P, cw], F32, tag="stg",
                                  name=f"stg_{ch}_{m}")
                for nn_ in range(2):
                    ps = psf.tile([P, VCH], F32, tag="psf",
                                  name=f"psf_{ch}_{m}_{nn_}")
                    for k in range(KD):
                        nc.tensor.matmul(
                            ps[:], xf_t[:, k, m * P:(m + 1) * P],
                            wt[:, k, nn_ * VCH:(nn_ + 1) * VCH],
                            start=(k == 0), stop=(k == KD - 1))
                    dsl = stg_t[:, nn_ * VCH:(nn_ + 1) * VCH]
                    if (m + nn_) % 2:
                        nc.scalar.copy(dsl, ps[:])
                    else:
                        nc.vector.tensor_copy(dsl, ps[:])
                nc.sync.dma_start(out_d[m * P:(m + 1) * P, c0:c0 + cw],
                                  stg_t[:])


def build_program():
    nc = bacc.Bacc("TRN2", target_bir_lowering=False, debug=False,
                   enable_asserts=False, num_devices=NCORE)

    def din(name, shape):
        return nc.dram_tensor(name, shape, F32, kind="ExternalInput").ap()

    t = {}
    t["x0T"] = din("x0T", (D, S))
    for nm in ("Wq", "Wk", "Wv", "Wo"):
        t[nm] = din(nm, (D, D))
    for nm in ("bq", "bk", "bv", "bo", "ln1g", "ln1b", "ln2g", "ln2b",
               "lnfg", "lnfb", "b2"):
        t[nm] = din(nm, (D, 1))
    t["W1"] = din("W1", (D, MLP))
    t["b1"] = din("b1", (MLP, 1))
    t["W2"] = din("W2", (MLP, D))
    t["qdT"] = din("qdT", (DH, H * C))
    t["cosT"] = din("cosT", (ROT, S))
    t["sinT"] = din("sinT", (ROT, S))
    t["RT"] = din("RT", (ROT, ROT))
    t["WoutS"] = din("WoutS", (D, VSL))
    t["out_d"] = nc.dram_tensor("out", (S, VSL), F32,
                                kind="ExternalOutput").ap()

    with tile.TileContext(nc) as tc, ExitStack() as ctx:
        _body(nc, tc, ctx, t)
    nc.compile()
    return nc


_STATE = {}


def _host_prep():
    inv = 1.0 / (THETA ** (np.arange(0, ROT, 2, dtype=np.float32) / ROT))
    ang = np.arange(S, dtype=np.float32)[:, None] * inv[None, :]  # (S, 16)
    cosT = np.repeat(np.cos(ang).T, 2, axis=0).astype(np.float32)  # (32, S)
    sinT = np.repeat(np.sin(ang).T, 2, axis=0).astype(np.float32)
    RT_ = np.zeros((ROT, ROT), np.float32)
    for i in range(ROT // 2):
        RT_[2 * i + 1, 2 * i] = -1.0
        RT_[2 * i, 2 * i + 1] = 1.0
    return cosT, sinT, RT_


def make_in_maps(tokens, emb, ln1_g, ln1_b, q_down, Wq, bq, Wk, bk, Wv, bv,
                 Wo, bo, ln2_g, ln2_b, W1, b1, W2, b2, lnf_g, lnf_b, Wout):
    cosT, sinT, RT_ = _host_prep()
    x0 = np.asarray(emb)[np.asarray(tokens)]  # (B, S, D)
    qd = np.asarray(q_down)[0].reshape(C, H, DH).transpose(2, 1, 0)
    qd = qd.reshape(DH, H * C)
    col = lambda a: np.asarray(a).reshape(-1, 1)
    common = {
        "Wq": Wq[0], "Wk": Wk[0], "Wv": Wv[0], "Wo": Wo[0],
        "bq": col(bq[0]), "bk": col(bk[0]), "bv": col(bv[0]),
        "bo": col(bo[0]),
        "ln1g": col(ln1_g[0]), "ln1b": col(ln1_b[0]),
        "ln2g": col(ln2_g[0]), "ln2b": col(ln2_b[0]),
        "lnfg": col(lnf_g), "lnfb": col(lnf_b),
        "W1": W1[0], "b1": col(b1[0]), "W2": W2[0], "b2": col(b2[0]),
        "qdT": qd, "cosT": cosT, "sinT": sinT, "RT": RT_,
    }
    common = {k: np.ascontiguousarray(v, dtype=np.float32)
              for k, v in common.items()}
    in_maps = []
    for c in range(NCORE):
        b, j = c // 4, c % 4
        m = dict(common)
        m["x0T"] = np.ascontiguousarray(np.asarray(x0[b]).T,
                                        dtype=np.float32)
        m["WoutS"] = np.ascontiguousarray(
            np.asarray(Wout)[:, j * VSL:(j + 1) * VSL], dtype=np.float32)
        in_maps.append(m)
    return in_maps


def kernel(**inputs):
    if "nc" not in _STATE:
        _STATE["nc"] = build_program()
    nc = _STATE["nc"]
    in_maps = make_in_maps(**inputs)
    res = run_bass_kernel_spmd(nc, in_maps, core_ids=list(range(NCORE)))
    out = np.empty((B, S, OUT), np.float32)
    for c in range(NCORE):
        b, j = c // 4, c % 4
        out[b, :, j * VSL:(j + 1) * VSL] = res.results[c]["out"]
    return out



# revision 9
# speedup vs baseline: 2.0719x; 2.0719x over previous
"""CompressionTransformer Trainium2 kernel.

Sharding (8 cores): core c -> batch b = c//4, vocab column block j = c%4.
Each core computes the full transformer body for its batch (1024 tokens)
and the final vocab projection for its 8000-column slice.  No collectives.

On-device layout: activations are kept feature-major ("transposed"):
xT[d, s] with d on partitions (tiles of 128) and tokens on the free dim.

All heavy matmuls run in bf16 (fp32 matmul is 4 cycles/row LOW_HIGH on
TRN2; bf16 is 1 cycle/row + fast weight load).  Accumulation stays fp32
in PSUM; layer-norm statistics, softmax normalizers and the linear-
attention state accumulator stay fp32 in SBUF.

Attention core = chunked linear attention over S in blocks of 128:
  state[d, c] = sum_{t < block} w[c,t] * vkv[t, d]   (SBUF accumulator)
with within-block terms computed via triangular-masked matmuls, and the
softmax normalizer cumsum done with the DVE prefix-scan instruction.
Max-subtraction is skipped in both softmaxes: it cancels exactly in the
compressed-KV ratio, and the up-logits are O(0.2) so exp() is safe.

Hardware constraint honored throughout: compute-engine operands must
share the same partition range (walrus `samePartitionsAll`); the only
partition-base shifts are done with DMA (head-1 projection split) or by
matmul PSUM output placement (odd-head attention output at base 64).
"""

import numpy as np
import ml_dtypes
from contextlib import ExitStack

import concourse.bass as bass
import concourse.mybir as mybir
import concourse.tile as tile
from concourse import bacc
from concourse.bass_utils import run_bass_kernel_spmd
from concourse.masks import make_identity, make_upper_triangular

F32 = mybir.dt.float32
BF = mybir.dt.bfloat16
AX = mybir.AluOpType
AF = mybir.ActivationFunctionType
AXL = mybir.AxisListType

B, S, D, H, C, V, MLP, OUT = 2, 1024, 512, 8, 64, 32000, 1024, 32000
DH = D // H          # 64
ROT = DH // 2        # 32
EPS = 1e-5
THETA = 10000.0
P = 128
KD = D // P          # 4  k-tiles over D
KM = MLP // P        # 8  k-tiles over MLP
NCORE = 8
VSL = OUT // 4       # 8000 per-core vocab slice
BLK = 128
NB = S // BLK        # 8
SCALE = DH ** -0.5
VCH = 500            # vocab columns per psum tile
NCH = VSL // (2 * VCH)  # 8 chunks of 1000 columns

_LN_ID = [0]


def _layer_norm(nc, tc, ps512, ones_col, ones_row, eps_ap, src, dst,
                g_cols, b_cols, r_tiles):
    """dst = LN(src) * g + b over the partition-tiled feature dim.
    src: fp32 (128, r_tiles, 1024) feature-major; dst may be bf16."""
    _LN_ID[0] += 1
    with tc.tile_pool(name=f"lnp{_LN_ID[0]}", bufs=1) as lnp:
        sumx = lnp.tile([1, S], F32, name="sumx")
        sumx2 = lnp.tile([1, S], F32, name="sumx2")
        tmp = lnp.tile([1, S], F32, name="lntmp")
        rstd = lnp.tile([1, S], F32, name="rstd")
        nmr = lnp.tile([1, S], F32, name="nmr")
        nc.vector.memset(sumx[:], 0.0)
        nc.vector.memset(sumx2[:], 0.0)
        for r in range(r_tiles):
            sq = lnp.tile([P, S], F32, tag="lnsq", name=f"lnsq{r}", bufs=2)
            nc.scalar.square(sq[:], src[:, r, :])
            for hh in range(2):
                sl = slice(hh * 512, hh * 512 + 512)
                ps_a = ps512.tile([1, 512], F32, tag="ps512",
                                  name=f"lps{r}{hh}a")
                nc.tensor.matmul(ps_a[:], ones_col[:], src[:, r, sl],
                                 start=True, stop=True)
                nc.vector.tensor_add(sumx[:, sl], sumx[:, sl], ps_a[:])
                ps_b = ps512.tile([1, 512], F32, tag="ps512",
                                  name=f"lps{r}{hh}b")
                nc.tensor.matmul(ps_b[:], ones_col[:], sq[:, sl],
                                 start=True, stop=True)
                nc.vector.tensor_add(sumx2[:, sl], sumx2[:, sl], ps_b[:])
        dd = float(r_tiles * P)
        nc.scalar.mul(sumx[:], sumx[:], 1.0 / dd)    # mean
        nc.scalar.mul(sumx2[:], sumx2[:], 1.0 / dd)  # E[x^2]
        nc.vector.tensor_mul(tmp[:], sumx[:], sumx[:])
        nc.vector.tensor_sub(tmp[:], sumx2[:], tmp[:])        # var
        nc.scalar.activation(tmp[:], tmp[:], AF.Sqrt, bias=eps_ap)
        nc.vector.reciprocal_approx_fast(rstd[:], tmp[:])
        nc.vector.scalar_tensor_tensor(nmr[:], sumx[:], -1.0, rstd[:],
                                       op0=AX.mult, op1=AX.mult)
        for hh in range(2):
            sl = slice(hh * 512, hh * 512 + 512)
            ps_r = ps512.tile([P, 512], F32, tag="ps512",
                              name=f"lbr{hh}")
            ps_n = ps512.tile([P, 512], F32, tag="ps512",
                              name=f"lbn{hh}")
            nc.tensor.matmul(ps_r[:], ones_row[:], rstd[:, sl], start=True,
                             stop=True)
            nc.tensor.matmul(ps_n[:], ones_row[:], nmr[:, sl], start=True,
                             stop=True)
            for r in range(r_tiles):
                t = dst[:, r, sl]
                nc.vector.tensor_mul(t, src[:, r, sl], ps_r[:])
                nc.vector.tensor_add(t, t, ps_n[:])
        for r in range(r_tiles):
            nc.scalar.activation(dst[:, r, :], dst[:, r, :], AF.Identity,
                                 bias=b_cols[:, r:r + 1],
                                 scale=g_cols[:, r:r + 1])


def _proj(nc, ps512, w_tile, rhs_tile, dst, bias_cols, m_tiles, k_tiles,
          resid=None):
    """dst (128, m_tiles, 1024) = W.T @ rhs + b [+ resid], feature-major."""
    for m in range(m_tiles):
        for hh in range(2):
            sl = slice(hh * 512, hh * 512 + 512)
            ps = ps512.tile([P, 512], F32, tag="ps512",
                            name=f"prj{m}{hh}")
            for k in range(k_tiles):
                nc.tensor.matmul(ps[:], w_tile[:, k, m * P:(m + 1) * P],
                                 rhs_tile[:, k, sl], start=(k == 0),
                                 stop=(k == k_tiles - 1))
            if resid is None:
                nc.scalar.activation(dst[:, m, sl], ps[:], AF.Identity,
                                     bias=bias_cols[:, m:m + 1], scale=1.0)
            else:
                nc.vector.scalar_tensor_tensor(
                    dst[:, m, sl], ps[:], bias_cols[:, m:m + 1],
                    resid[:, m, sl], op0=AX.add, op1=AX.add)


def _attn_head(nc, hsc, sb, ps512, pta, ptsm, ident, ident_bf, ut, qdT_s,
               qu_p, kd_0, vk_p, vv_p, po, hd, outT):
    """One compression-attention head.  qu/vk/vv (bf16) are used as
    pair-tile slices at partition base po (matmul inputs only -- their
    PSUM results land at base 0); kd_0 is a base-0 (64, S) bf16
    view/copy.  All other per-head tensors live at base 0.  For po=64
    the K/V column order inside vkv_tok/state is swapped so stateK sits
    at rows [64:128], matching qu's partition base in the logits
    matmul."""
    hsl = slice(po, po + DH)             # this head's rows in pair tiles
    kcol = slice(po, po + DH)            # K half inside (t, d) tiles
    vcol = slice((po + DH) % P, (po + DH) % P + DH)  # V half
    idq = ident_bf[hsl, hsl]             # 64x64 bf16 identity at base po
    id0 = ident[0:DH, 0:DH]
    id0_bf = ident_bf[0:DH, 0:DH]
    # down[c, s] = qd_scaled[:, c] . kd_rope[:, s]
    # w = exp(down): |down| << 1, and any per-c max shift cancels exactly
    # in the kvu/norm ratio, so no max-subtraction is needed.
    w_cs = hsc.tile([C, S], F32, tag="wcs", name=f"wcs{hd}")
    for hh in range(2):
        sl = slice(hh * 512, hh * 512 + 512)
        down_ps = ps512.tile([C, 512], F32, tag="ps512",
                             name=f"dwn{hd}_{hh}")
        nc.tensor.matmul(down_ps[:], qdT_s[:, hd * C:(hd + 1) * C],
                         kd_0[:, sl], start=True, stop=True)
        nc.scalar.activation(w_cs[:, sl], down_ps[:], AF.Exp, bias=0.0,
                             scale=1.0)
    w_bf = hsc.tile([C, S], BF, tag="wbf", name=f"wbf{hd}")
    nc.vector.tensor_copy(w_bf[:], w_cs[:])
    # inclusive cumsum over s: state=(w+state) max w  (w>0 so max is id)
    ncs = hsc.tile([C, S], F32, tag="ncs", name=f"ncs{hd}")
    nc.vector.tensor_tensor_scan(ncs[:], w_cs[:], w_cs[:], 0.0,
                                 op0=AX.add, op1=AX.max)
    nci = hsc.tile([C, S], F32, tag="nci", name=f"nci{hd}")
    nc.vector.reciprocal_approx_fast(nci[:], ncs[:])

    state = sb.tile([P, C], F32, tag="state", name=f"st{hd}")  # (2DH, C)
    nc.vector.memset(state[:], 0.0)
    state_bf = sb.tile([P, C], BF, tag="statebf", name=f"stb{hd}")
    nc.vector.memset(state_bf[:], 0.0)

    otile = (hd * DH) // P
    if po:
        oT_h = hsc.tile([DH, S], BF, tag="oTh", name=f"oTh{hd}")

    for n in range(NB):
        bsl = slice(n * BLK, (n + 1) * BLK)
        # vkv_tok (t, d): transposed vk/vv block halves (K at kcol)
        vkvP = pta.tile([P, P], BF, tag="ptabf", name=f"vkvP{hd}_{n}")
        nc.tensor.transpose(vkvP[:, kcol], vk_p[hsl, bsl], idq)
        nc.tensor.transpose(vkvP[:, vcol], vv_p[hsl, bsl], idq)
        vkv_tok = sb.tile([P, P], BF, tag="vkvtok", name=f"vkv{hd}_{n}")
        nc.scalar.copy(vkv_tok[:], vkvP[:])
        # w_tok (t, c)
        wtP = ptsm.tile([P, C], F32, tag="ptsm", name=f"wtP{hd}_{n}")
        nc.tensor.transpose(wtP[:], w_cs[:, bsl], id0)
        w_tok = sb.tile([P, C], BF, tag="wtok", name=f"wtk{hd}_{n}")
        nc.vector.tensor_copy(w_tok[:], wtP[:])
        # invn_T (t, c)
        invP = ptsm.tile([P, C], F32, tag="ptsm", name=f"invP{hd}_{n}")
        nc.tensor.transpose(invP[:], nci[:, bsl], id0)
        invn_T = sb.tile([P, C], F32, tag="invnT", name=f"invT{hd}_{n}")
        nc.scalar.copy(invn_T[:], invP[:])
        # A_T[t,i] = vk[t] . qu[i], keep t <= i
        atP = pta.tile([P, P], F32, tag="pta", name=f"atP{hd}_{n}")
        nc.tensor.matmul(atP[:], vk_p[hsl, bsl], qu_p[hsl, bsl],
                         start=True, stop=True)
        at_m = sb.tile([P, P], BF, tag="atm", name=f"atm{hd}_{n}")
        nc.vector.tensor_mul(at_m[:], atP[:], ut[:])
        # state snapshot (c, d) at base 0
        cdP = ptsm.tile([C, P], F32, tag="ptsm", name=f"cdP{hd}_{n}")
        nc.tensor.transpose(cdP[:], state[:], ident[:])
        st_cd = sb.tile([C, P], BF, tag="stcd", name=f"stcd{hd}_{n}")
        nc.scalar.copy(st_cd[:], cdP[:])
        # logits(i, c) = qu . stateK + within-block
        lgP = ptsm.tile([P, C], F32, tag="ptsm", name=f"lgP{hd}_{n}")
        nc.tensor.matmul(lgP[:], qu_p[hsl, bsl], state_bf[kcol, :],
                         start=True, stop=False)
        nc.tensor.matmul(lgP[:], at_m[:], w_tok[:], start=False, stop=True)
        lg = sb.tile([P, C], F32, tag="lg", name=f"lg{hd}_{n}")
        nc.vector.scalar_tensor_tensor(lg[:], lgP[:], SCALE, invn_T[:],
                                       op0=AX.mult, op1=AX.mult)
        # softmax over c (free dim); logits are O(0.2) so skip the max
        pexp = sb.tile([P, C], F32, tag="pexp", name=f"pex{hd}_{n}")
        ssum = sb.tile([P, 1], F32, tag="ssum", name=f"ssum{hd}_{n}")
        nc.scalar.activation(pexp[:], lg[:], AF.Exp, bias=0.0,
                             scale=1.0, accum_out=ssum[:])
        rsum = sb.tile([P, 1], F32, tag="rsum", name=f"rsum{hd}_{n}")
        nc.vector.reciprocal(rsum[:], ssum[:])
        r_sb = sb.tile([P, C], F32, tag="rsb", name=f"rsb{hd}_{n}")
        nc.vector.tensor_scalar_mul(r_sb[:], pexp[:], rsum[:])
        nc.vector.tensor_mul(r_sb[:], r_sb[:], invn_T[:])
        # rT (c, i) at base 0
        rtP = ptsm.tile([C, P], F32, tag="ptsm", name=f"rtP{hd}_{n}")
        nc.tensor.transpose(rtP[:], r_sb[:], ident[:])
        rT = sb.tile([C, P], BF, tag="rt", name=f"rT{hd}_{n}")
        nc.scalar.copy(rT[:], rtP[:])
        # G_T[t,i] = sum_c w[c,t] rT[c,i], keep t <= i
        gtP = pta.tile([P, P], F32, tag="pta", name=f"gtP{hd}_{n}")
        nc.tensor.matmul(gtP[:], w_bf[:, bsl], rT[:], start=True,
                         stop=True)
        gt_m = sb.tile([P, P], BF, tag="gtm", name=f"gtm{hd}_{n}")
        nc.vector.tensor_mul(gt_m[:], gtP[:], ut[:])
        # out block (dv, i) at psum base 0
        outP = ptsm.tile([DH, P], F32, tag="ptsm", name=f"outP{hd}_{n}")
        nc.tensor.matmul(outP[:], st_cd[:, vcol], rT[:], start=True,
                         stop=False)
        nc.tensor.matmul(outP[:], vkv_tok[:, vcol], gt_m[:], start=False,
                         stop=True)
        if po:
            nc.scalar.copy(oT_h[:, bsl], outP[:])
        else:
            nc.scalar.copy(outT[0:DH, otile, bsl], outP[:])
        # state += vkv_tok.T @ w_tok
        sdP = pta.tile([P, C], F32, tag="pta", name=f"sdP{hd}_{n}")
        nc.tensor.matmul(sdP[:], vkv_tok[:], w_tok[:], start=True,
                         stop=True)
        nc.vector.tensor_add(state[:], state[:], sdP[:])
        nc.scalar.copy(state_bf[:], state[:])

    if po:
        # partition shift rows [0:64] -> [64:128] via SBUF-to-SBUF DMA
        nc.sync.dma_start(outT[DH:P, otile, :], oT_h[:])


def _body(nc, tc, ctx, t):
    x0T, Wq, Wk, Wv, Wo = t["x0T"], t["Wq"], t["Wk"], t["Wv"], t["Wo"]
    W1, W2, WoutS, out_d = t["W1"], t["W2"], t["WoutS"], t["out_d"]
    qdT, cosT, sinT, RT = t["qdT"], t["cosT"], t["sinT"], t["RT"]

    const = ctx.enter_context(tc.tile_pool(name="const", bufs=1))
    acts = ctx.enter_context(tc.tile_pool(name="acts", bufs=1))
    sb = ctx.enter_context(tc.tile_pool(name="sb", bufs=2))

    # ---- constants ----
    ident = const.tile([P, P], F32)
    make_identity(nc, ident[:])
    ident_bf = const.tile([P, P], BF)
    nc.vector.tensor_copy(ident_bf[:], ident[:])
    ut = const.tile([P, P], F32)
    make_upper_triangular(nc, ut[:], val=1.0, diag=True)
    ones_col = const.tile([P, 1], F32)
    nc.vector.memset(ones_col[:], 1.0)
    ones_row = const.tile([1, P], F32)
    nc.vector.memset(ones_row[:], 1.0)
    eps_t = const.tile([1, 1], F32)
    nc.vector.memset(eps_t[:], EPS)

    cosT_t = const.tile([ROT, S], BF)
    nc.sync.dma_start(cosT_t[:], cosT)
    sinT_t = const.tile([ROT, S], BF)
    nc.sync.dma_start(sinT_t[:], sinT)
    RT_t = const.tile([ROT, ROT], BF)
    nc.sync.dma_start(RT_t[:], RT)
    qdT_s = const.tile([DH, H * C], BF)
    nc.sync.dma_start(qdT_s[:], qdT)
    nc.scalar.mul(qdT_s[:], qdT_s[:], SCALE)

    def load_col(ap, kt, name, p=P):
        c_ = const.tile([p, kt], F32, name=name)
        nc.sync.dma_start(c_[:], ap.rearrange("(t p) o -> p (t o)", p=p))
        return c_

    # full-width (128 x KD) bias/gain columns
    bo_c = load_col(t["bo"], KD, "bo_c")
    b1_c = load_col(t["b1"], KM, "b1_c")
    b2_c = load_col(t["b2"], KD, "b2_c")
    ln1g_c = load_col(t["ln1g"], KD, "ln1g_c")
    ln1b_c = load_col(t["ln1b"], KD, "ln1b_c")
    ln2g_c = load_col(t["ln2g"], KD, "ln2g_c")
    ln2b_c = load_col(t["ln2b"], KD, "ln2b_c")
    lnfg_c = load_col(t["lnfg"], KD, "lnfg_c")
    lnfb_c = load_col(t["lnfb"], KD, "lnfb_c")
    bv_c = load_col(t["bv"], KD, "bv_c")
    bq_c = load_col(t["bq"], KD, "bq_c")
    bk_c = load_col(t["bk"], KD, "bk_c")

    x0_t = acts.tile([P, KD, S], F32, tag="A")
    nc.sync.dma_start(x0_t[:], x0T.rearrange("(t p) n -> p t n", p=P))

    with tc.tile_pool(name="wmain", bufs=1) as wmain:
        def load_w(ap, kt, n, name):
            w = wmain.tile([P, kt, n], BF, name=name)
            nc.sync.dma_start(w[:], ap.rearrange("(t p) n -> p t n", p=P))
            return w

        Wq_t = load_w(Wq, KD, D, "Wq_t")
        Wk_t = load_w(Wk, KD, D, "Wk_t")
        Wv_t = load_w(Wv, KD, D, "Wv_t")
        Wo_t = load_w(Wo, KD, D, "Wo_t")

        with tc.tile_pool(name="ps512", bufs=2, space="PSUM") as ps512:
            # ---- LN1 ----
            h_t = acts.tile([P, KD, S], BF, tag="B")
            _layer_norm(nc, tc, ps512, ones_col, ones_row, eps_t[:], x0_t,
                        h_t, ln1g_c, ln1b_c, KD)

            # ---- vd = h @ Wv + bv ----
            vd_t = acts.tile([P, KD, S], BF, tag="C")
            _proj(nc, ps512, Wv_t, h_t, vd_t, bv_c, KD, KD)

            outT = acts.tile([P, KD, S], BF, tag="D")

            # ---- head pairs: projections + rope + attention ----
            with tc.tile_pool(name="hsc", bufs=2) as hsc, \
                 tc.tile_pool(name="pta", bufs=2, space="PSUM") as pta, \
                 tc.tile_pool(name="ptsm", bufs=2, space="PSUM") as ptsm:
                for pr in range(4):
                    msl = slice(pr * P, (pr + 1) * P)
                    ht = {k: hsc.tile([P, S], BF, tag=f"hd_{k}",
                                      name=f"{k}_{pr}")
                          for k in ("qu", "kd", "vk", "vv")}
                    for ky, wt, bcol, rhs in (("qu", Wq_t, bq_c, h_t),
                                              ("kd", Wk_t, bk_c, h_t),
                                              ("vk", Wk_t, bk_c, vd_t),
                                              ("vv", Wv_t, bv_c, vd_t)):
                        for hh in range(2):
                            sl = slice(hh * 512, hh * 512 + 512)
                            ps = ps512.tile([P, 512], F32, tag="ps512",
                                            name=f"pp{pr}{ky}{hh}")
                            for k in range(KD):
                                nc.tensor.matmul(ps[:], wt[:, k, msl],
                                                 rhs[:, k, sl],
                                                 start=(k == 0),
                                                 stop=(k == KD - 1))
                            if hh % 2:
                                nc.scalar.activation(
                                    ht[ky][:, sl], ps[:], AF.Identity,
                                    bias=bcol[:, pr:pr + 1], scale=1.0)
                            else:
                                nc.vector.tensor_scalar_add(
                                    ht[ky][:, sl], ps[:],
                                    bcol[:, pr:pr + 1])
                    # per-head base-0 kd (odd head: DMA partition shift),
                    # then rope on its first ROT dims
                    kd0s = []
                    for hi in range(2):
                        if hi == 0:
                            kd_0 = ht["kd"][0:DH, :]
                        else:
                            kd_0t = hsc.tile([DH, S], BF, tag="kd0",
                                             name=f"kd0_{pr}")
                            nc.sync.dma_start(kd_0t[:], ht["kd"][DH:P, :])
                            kd_0 = kd_0t[:]
                        kd0s.append(kd_0)
                        tmp = hsc.tile([ROT, S], BF, tag="ropet",
                                       name=f"ropes_{pr}_{hi}")
                        nc.vector.tensor_mul(tmp[:], kd_0[0:ROT, :],
                                             sinT_t[:])
                        rp = ps512.tile([ROT, 512], F32, tag="ps512",
                                        name=f"rp{pr}{hi}")
                        rp2 = ps512.tile([ROT, 512], F32, tag="ps512",
                                         name=f"rq{pr}{hi}")
                        nc.tensor.matmul(rp[:], RT_t[:], tmp[:, 0:512],
                                         start=True, stop=True)
                        nc.tensor.matmul(rp2[:], RT_t[:], tmp[:, 512:1024],
                                         start=True, stop=True)
                        tmp2 = hsc.tile([ROT, S], BF, tag="ropet",
                                        name=f"ropec_{pr}_{hi}")
                        nc.vector.tensor_mul(tmp2[:], kd_0[0:ROT, :],
                                             cosT_t[:])
                        nc.vector.tensor_add(kd_0[0:ROT, 0:512],
                                             tmp2[:, 0:512], rp[:])
                        nc.vector.tensor_add(kd_0[0:ROT, 512:1024],
                                             tmp2[:, 512:1024], rp2[:])

                    for hi in range(2):
                        _attn_head(nc, hsc, sb, ps512, pta, ptsm, ident,
                                   ident_bf, ut, qdT_s, ht["qu"], kd0s[hi],
                                   ht["vk"], ht["vv"], hi * DH,
                                   2 * pr + hi, outT)

            # ---- a = out @ Wo + bo ; resid = h + a ----
            resid = acts.tile([P, KD, S], F32, tag="A")
            _proj(nc, ps512, Wo_t, outT, resid, bo_c, KD, KD, resid=h_t)

            # ---- x2 = LN2(resid) ----
            x2_t = acts.tile([P, KD, S], BF, tag="B")
            _layer_norm(nc, tc, ps512, ones_col, ones_row, eps_t[:], resid,
                        x2_t, ln2g_c, ln2b_c, KD)

            # ---- MLP: x3 = x2 + relu(x2@W1+b1)@W2 + b2 ----
            x3_t = acts.tile([P, KD, S], F32, tag="A")
            w1_r = W1.rearrange("(t p) n -> p t n", p=P)
            with tc.tile_pool(name="wmlp", bufs=2) as wmlp, \
                 tc.tile_pool(name="psacc", bufs=4, space="PSUM") as psacc:
                for hh in range(2):
                    sl = slice(hh * 512, hh * 512 + 512)
                    m2ps = [psacc.tile([P, 512], F32, tag="m2ps",
                                       name=f"m2ps_{hh}_{i}")
                            for i in range(KD)]
                    for km in range(KM):
                        w1t = wmlp.tile([P, KD, P], BF, tag="w1t",
                                        name=f"w1t_{hh}_{km}")
                        nc.sync.dma_start(
                            w1t[:], w1_r[:, :, km * P:(km + 1) * P])
                        w2t = wmlp.tile([P, D], BF, tag="w2t",
                                        name=f"w2t_{hh}_{km}")
                        nc.sync.dma_start(w2t[:],
                                          W2[km * P:(km + 1) * P, :])
                        m1ps = ps512.tile([P, 512], F32, tag="ps512",
                                          name=f"m1ps_{hh}_{km}")
                        for k in range(KD):
                            nc.tensor.matmul(m1ps[:], w1t[:, k, :],
                                             x2_t[:, k, sl], start=(k == 0),
                                             stop=(k == KD - 1))
                        m1sb = sb.tile([P, 512], BF, tag="m1sb",
                                       name=f"m1sb_{hh}_{km}")
                        nc.scalar.activation(m1sb[:], m1ps[:], AF.Relu,
                                             bias=b1_c[:, km:km + 1],
                                             scale=1.0)
                        for m in range(KD):
                            nc.tensor.matmul(m2ps[m][:],
                                             w2t[:, m * P:(m + 1) * P],
                                             m1sb[:], start=(km == 0),
                                             stop=(km == KM - 1))
                    for m in range(KD):
                        nc.vector.scalar_tensor_tensor(
                            x3_t[:, m, sl], m2ps[m][:], b2_c[:, m:m + 1],
                            x2_t[:, m, sl], op0=AX.add, op1=AX.add)

            # ---- xf = LNf(x3) ----
            xf_t = acts.tile([P, KD, S], BF, tag="C")
            _layer_norm(nc, tc, ps512, ones_col, ones_row, eps_t[:], x3_t,
                        xf_t, lnfg_c, lnfb_c, KD)

    # ---- final projection: out = xf @ WoutS ----
    with tc.tile_pool(name="wout", bufs=2) as woutp, \
         tc.tile_pool(name="stg", bufs=3) as stgp, \
         tc.tile_pool(name="psf", bufs=4, space="PSUM") as psf:
        wout_r = WoutS.rearrange("(t p) n -> p t n", p=P)
        cw = 2 * VCH
        for ch in range(NCH):
            c0 = ch * cw
            wt = woutp.tile([P, KD, cw], BF, tag="woutt",
                            name=f"wout_{ch}")
            nc.sync.dma_start(wt[:], wout_r[:, :, c0:c0 + cw])
            for m in range(8):
                stg_t = stgp.tile([P, cw], F32, tag="stg",
                                  name=f"stg_{ch}_{m}")
                for nn_ in range(2):
                    ps = psf.tile([P, VCH], F32, tag="psf",
                                  name=f"psf_{ch}_{m}_{nn_}")
                    for k in range(KD):
                        nc.tensor.matmul(
                            ps[:], xf_t[:, k, m * P:(m + 1) * P],
                            wt[:, k, nn_ * VCH:(nn_ + 1) * VCH],
                            start=(k == 0), stop=(k == KD - 1))
                    dsl = stg_t[:, nn_ * VCH:(nn_ + 1) * VCH]
                    if (m + nn_) % 2:
                        nc.scalar.copy(dsl, ps[:])
                    else:
                        nc.vector.tensor_copy(dsl, ps[:])
                nc.sync.dma_start(out_d[m * P:(m + 1) * P, c0:c0 + cw],
                                  stg_t[:])


def build_program():
    nc = bacc.Bacc("TRN2", target_bir_lowering=False, debug=False,
                   enable_asserts=False, num_devices=NCORE)

    def din(name, shape, dt=F32):
        return nc.dram_tensor(name, shape, dt, kind="ExternalInput").ap()

    t = {}
    t["x0T"] = din("x0T", (D, S))
    for nm in ("Wq", "Wk", "Wv", "Wo"):
        t[nm] = din(nm, (D, D), BF)
    for nm in ("bq", "bk", "bv", "bo", "ln1g", "ln1b", "ln2g", "ln2b",
               "lnfg", "lnfb", "b2"):
        t[nm] = din(nm, (D, 1))
    t["W1"] = din("W1", (D, MLP), BF)
    t["b1"] = din("b1", (MLP, 1))
    t["W2"] = din("W2", (MLP, D), BF)
    t["qdT"] = din("qdT", (DH, H * C), BF)
    t["cosT"] = din("cosT", (ROT, S), BF)
    t["sinT"] = din("sinT", (ROT, S), BF)
    t["RT"] = din("RT", (ROT, ROT), BF)
    t["WoutS"] = din("WoutS", (D, VSL), BF)
    t["out_d"] = nc.dram_tensor("out", (S, VSL), F32,
                                kind="ExternalOutput").ap()

    with tile.TileContext(nc) as tc, ExitStack() as ctx:
        _body(nc, tc, ctx, t)
    nc.compile()
    return nc


_STATE = {}


def _host_prep():
    inv = 1.0 / (THETA ** (np.arange(0, ROT, 2, dtype=np.float32) / ROT))
    ang = np.arange(S, dtype=np.float32)[:, None] * inv[None, :]  # (S, 16)
    cosT = np.repeat(np.cos(ang).T, 2, axis=0).astype(np.float32)  # (32, S)
    sinT = np.repeat(np.sin(ang).T, 2, axis=0).astype(np.float32)
    RT_ = np.zeros((ROT, ROT), np.float32)
    for i in range(ROT // 2):
        RT_[2 * i + 1, 2 * i] = -1.0
        RT_[2 * i, 2 * i + 1] = 1.0
    return cosT, sinT, RT_


def make_in_maps(tokens, emb, ln1_g, ln1_b, q_down, Wq, bq, Wk, bk, Wv, bv,
                 Wo, bo, ln2_g, ln2_b, W1, b1, W2, b2, lnf_g, lnf_b, Wout):
    cosT, sinT, RT_ = _host_prep()
    x0 = np.asarray(emb)[np.asarray(tokens)]  # (B, S, D)
    qd = np.asarray(q_down)[0].reshape(C, H, DH).transpose(2, 1, 0)
    qd = qd.reshape(DH, H * C)
    col = lambda a: np.asarray(a).reshape(-1, 1)
    BFN = ml_dtypes.bfloat16
    common = {
        "Wq": Wq[0], "Wk": Wk[0], "Wv": Wv[0], "Wo": Wo[0],
        "bq": col(bq[0]), "bk": col(bk[0]), "bv": col(bv[0]),
        "bo": col(bo[0]),
        "ln1g": col(ln1_g[0]), "ln1b": col(ln1_b[0]),
        "ln2g": col(ln2_g[0]), "ln2b": col(ln2_b[0]),
        "lnfg": col(lnf_g), "lnfb": col(lnf_b),
        "W1": W1[0], "b1": col(b1[0]), "W2": W2[0], "b2": col(b2[0]),
        "qdT": qd, "cosT": cosT, "sinT": sinT, "RT": RT_,
    }
    bf_names = {"Wq", "Wk", "Wv", "Wo", "W1", "W2", "qdT", "cosT", "sinT",
                "RT"}
    common = {k: np.ascontiguousarray(v, dtype=BFN if k in bf_names
                                      else np.float32)
              for k, v in common.items()}
    in_maps = []
    for c in range(NCORE):
        b, j = c // 4, c % 4
        m = dict(common)
        m["x0T"] = np.ascontiguousarray(np.asarray(x0[b]).T,
                                        dtype=np.float32)
        m["WoutS"] = np.ascontiguousarray(
            np.asarray(Wout)[:, j * VSL:(j + 1) * VSL], dtype=BFN)
        in_maps.append(m)
    return in_maps


def kernel(**inputs):
    if "nc" not in _STATE:
        _STATE["nc"] = build_program()
    nc = _STATE["nc"]
    in_maps = make_in_maps(**inputs)
    res = run_bass_kernel_spmd(nc, in_maps, core_ids=list(range(NCORE)))
    out = np.empty((B, S, OUT), np.float32)
    for c in range(NCORE):
        b, j = c // 4, c % 4
        out[b, :, j * VSL:(j + 1) * VSL] = res.results[c]["out"]
    return out
